# revision 36
# baseline (speedup 1.0000x reference)
"""Trainium2 Bass kernel for nn_CausalAttention (B=2, T=2048, C=2048, H=16, ALiBi).

Sharding: 8 cores = 2 (batch) x 4 (head groups). Core c handles batch c//4 and
heads [g, g+4, g+8, g+12] where g = c%4 (strided so the ALiBi slope mix is
balanced across cores). One SPMD program; every slope-dependent value enters
as data (exp-bias table, query-shift rows), never as a program constant.

All matmul operands are bf16 (fp32 PSUM accumulation): rel err ~3e-3 on the
final output, well inside the gate, and it halves DMA/SBUF and enables the
PE fast-weight-load path. Everything is SBUF-resident; the only HBM traffic
is the inputs (x^T + weights, bf16) and the fp32 partial-output store.

Per-core device pipeline:
  A) qT/kT [d,t] and v [t,d] projections from host-pretransposed x^T, streamed
     by 512-wide t-slices, weights and x slices arriving kc-chunked on four
     parallel DMA queues so the first matmul unblocks within ~1us. Wq is
     host-prescaled by 1/sqrt(D). All of qT/kT/v stays in SBUF (bf16).
  B) Per query chunk j (descending, biggest first), per head: S^T[tk,tq] =
     kT.T @ qT in PSUM. ALiBi enters as (i) an exact fp32 per-partition
     exp-bias column from a host table (key-side ramp; 1024-centred for the
     two steep head positions, chunk-end-centred for the shallow two) and
     (ii) for the steep positions a query-side shift row folded in by a
     rank-1 matmul PSUM preload (softmax-invariant; range control only).
     ACT computes E = exp(.) into SBUF bf16; GPSIMD masks diagonal tiles
     (affine_select, fill 0). PV and the denominator both accumulate on the
     PE (den via an all-ones stationary, output pre-broadcast across
     partitions), so no vector-engine reduction chain exists. DVE only does
     the reciprocal + normalize per (head, chunk). Diagonal tiles compute
     only the live column range. Far tiles with slope*(tq-tk) >= 150
     everywhere are skipped (exp underflows to 0 in the fp32 reference too).
  C) Interleaved per chunk j, right after its 4 heads: out[t,c] partial =
     sum_h O_norm_h^T.T @ Wo_h from SBUF, stores fanned over all four DMA
     queues. Host sums the 4 head-group partials per batch and adds bo.
Key bias bk cancels in softmax; bq/bv (zero in practice) are otherwise
added on-device via K=1 outer-product matmuls.
"""

import math
import sys

sys.path.insert(0, "/opt/trn_rl_repo")

import numpy as np
import ml_dtypes

import concourse.mybir as mybir  # noqa: E402
import concourse.tile as tile  # noqa: E402
from concourse import bacc  # noqa: E402
from concourse.bass_utils import run_bass_kernel_spmd  # noqa: E402

B, T, C, H = 2, 2048, 2048, 16
D = C // H  # 128
P = 128
NKC = C // P       # 16 contraction tiles
NKT = T // P       # 16 key tiles
NQC = T // 512     # 4 query chunks of 512
HPG = 4            # heads per core
SQD = math.sqrt(D)
SKIP_CUT = 40.0  # skipped tiles have softmax weight <= e^-30: far below the gate
F32 = mybir.dt.float32
BF16 = mybir.dt.bfloat16
EXP = mybir.ActivationFunctionType.Exp
BF = ml_dtypes.bfloat16


def _slopes(n=16):
    start = 2.0 ** (-2.0 ** -(math.log2(n) - 3))
    return [start * start**i for i in range(n)]


SLOPES = _slopes(H)


def _core_heads(g):
    return [g, g + 4, g + 8, g + 12]


def _kts_for_chunk(hi, j):
    # Union over cores: the smallest slope in head-position hi is head 4*hi+3.
    s = SLOPES[4 * hi + 3]
    out = []
    for kt in range(4 * j + 4):
        mind = 512 * j - 128 * kt - 127
        if s * mind < SKIP_CUT:
            out.append(kt)
    return out


_PROG_CACHE = {}


def _build_program(use_b):
    if use_b in _PROG_CACHE:
        return _PROG_CACHE[use_b]
    use_bq, use_bk, use_bv = use_b

    nc = bacc.Bacc(None)
    xt_d = nc.declare_dram_parameter("xt", [C, T], BF16, isOutput=False)
    wq_d = nc.declare_dram_parameter("wq", [C, HPG * D], BF16, isOutput=False)
    wk_d = nc.declare_dram_parameter("wk", [C, HPG * D], BF16, isOutput=False)
    wv_d = nc.declare_dram_parameter("wv", [C, HPG * D], BF16, isOutput=False)
    wo_d = nc.declare_dram_parameter("wo", [HPG * D, C], BF16, isOutput=False)
    qrow_d = nc.declare_dram_parameter("qrow", [1, HPG * NQC * 512], BF16, isOutput=False)
    ktab_d = nc.declare_dram_parameter("ktab", [P, HPG * NQC * NKT], F32, isOutput=False)
    ones_d = nc.declare_dram_parameter("ones", [P, 512], BF16, isOutput=False)
    # causal-mask matmul constants: step[k,p] = -1000*[k<p]; wide one-hot
    # wide[k,g] = [g == k+384]. step.T @ wide[:, 384:896-off] adds -1000 on
    # the masked triangle of a diagonal S tile, so exp gives exact zeros.
    step_d = nc.declare_dram_parameter("step", [P, P], BF16, isOutput=False)
    wide_d = nc.declare_dram_parameter("wide", [P, 896], BF16, isOutput=False)
    if any(use_b):
        bqkv_d = nc.declare_dram_parameter("bqkv", [3, HPG * D], BF16, isOutput=False)
        onesrow_d = nc.declare_dram_parameter("onesrow", [1, 512], BF16, isOutput=False)
    y_d = nc.declare_dram_parameter("y", [T, C], BF16, isOutput=True)

    with tile.TileContext(nc) as tc:
        with (
            tc.tile_pool(name="perm", bufs=1) as perm,
            tc.tile_pool(name="dram", bufs=1, space="DRAM") as dpool,
        ):
            ones_sb = perm.tile([P, 512], BF16, tag="ones")
            nc.sync.dma_start(ones_sb[:], ones_d[:])
            step_sb = perm.tile([P, P], BF16, tag="step")
            nc.sync.dma_start(step_sb[:], step_d[:])
            wide_sb = perm.tile([P, 896], BF16, tag="wide")
            nc.sync.dma_start(wide_sb[:], wide_d[:])
            ktab_sb = perm.tile([P, HPG, NQC, NKT], F32, tag="ktab")
            nc.sync.dma_start(
                ktab_sb[:],
                ktab_d[:].rearrange("p (h j k) -> p h j k", h=HPG, j=NQC),
            )
            qrow_sb = perm.tile([1, HPG, NQC, 512], BF16, tag="qrow")
            nc.sync.dma_start(
                qrow_sb[:],
                qrow_d[:].rearrange("o (h j f) -> o h j f", h=HPG, j=NQC),
            )
            if any(use_b):
                bqkv_sb = perm.tile([3, HPG * D], BF16, tag="bqkv")
                onesrow_sb = perm.tile([1, 512], BF16, tag="onesrow")
                nc.sync.dma_start(bqkv_sb[:], bqkv_d[:])
                nc.sync.dma_start(onesrow_sb[:], onesrow_d[:])

            # SBUF-resident projections + attention outputs (bf16).
            qt_all = perm.tile([P, HPG, T], BF16, tag="qt")
            kt_all = perm.tile([P, HPG, T], BF16, tag="kt")
            v_all = perm.tile([P, NKT, HPG * D], BF16, tag="v")
            on_all = perm.tile([P, HPG, T], BF16, tag="on")
            wo_sb = perm.tile([P, HPG, C], BF16, tag="wo")

            # ---------------- Phase A: projections ----------------
            with (
                tc.tile_pool(name="xtp", bufs=2) as xtp,
                tc.tile_pool(name="wp", bufs=1) as wp,
                tc.tile_pool(name="psA", bufs=8, space="PSUM") as psA,
            ):
                wq_sb = wp.tile([P, NKC, HPG * D], BF16, tag="wq")
                wk_sb = wp.tile([P, NKC, HPG * D], BF16, tag="wk")
                wv_sb = wp.tile([P, NKC, HPG * D], BF16, tag="wv")
                # The scalar+sync queues share one HWDGE ring set (~190GB/s),
                # gpsimd drives SWDGE (~150GB/s); ring order is issue order.
                # Interleave wq/xt(tn0) chunk pairs so the first Q chain's
                # inputs land together, splitting the tail onto SWDGE; then
                # queue the rest on SWDGE in need-order (wk, wv, xt1-3, wo).
                xt0_sb = xtp.tile([P, NKC, 512], BF16, tag="xt")
                for kc in range(NKC):
                    eng = nc.scalar if kc < 10 else nc.gpsimd
                    eng.dma_start(wq_sb[:, kc, :], wq_d[kc * P:(kc + 1) * P, :])
                    eng.dma_start(xt0_sb[:, kc, :], xt_d[kc * P:(kc + 1) * P, 0:512])
                for kc in range(NKC):
                    # low kc on the HWDGE stream right behind the pairs; high
                    # kc leads SWDGE so the K accumulation chains never wait
                    eng = nc.scalar if kc < 8 else nc.gpsimd
                    eng.dma_start(wk_sb[:, kc, :], wk_d[kc * P:(kc + 1) * P, :])
                for kc in range(NKC):
                    nc.gpsimd.dma_start(
                        wv_sb[:, kc, :], wv_d[kc * P:(kc + 1) * P, :]
                    )

                # PE warm-up across the initial DMA window (HAM reaches
                # K=8/8 before the projection chains start), doubling as a
                # microbench: 16 N=512 then 16 N=1024 bf16 matmuls.
                wb_ps = psA.tile([P, 512], F32, tag="pp")
                for wi in range(24):
                    nc.tensor.matmul(
                        wb_ps[:], ones_sb[:, :P], ones_sb[:],
                        start=True, stop=True,
                    )
                warm_out = wp.tile([P, 512], F32, tag="wout")
                nc.vector.tensor_copy(warm_out[:], wb_ps[:])
                warm_d = dpool.tile([P, 512], F32, tag="warmd", name="warm_d")
                nc.sync.dma_start(warm_d[:], warm_out[:])

                for tn in range(NQC):
                    ts = slice(tn * 512, (tn + 1) * 512)
                    if tn == 0:
                        xt_sb = xt0_sb
                        # consume kc in DMA arrival order: SWDGE chunks land
                        # first, then the HWDGE stream, so the four
                        # interleaved chains chase the transfers without gaps
                        qorder = list(range(10, NKC)) + list(range(10))
                        korder = list(range(8, NKC)) + list(range(8))
                    else:
                        xt_sb = xtp.tile([P, NKC, 512], BF16, tag="xt")
                        nc.gpsimd.dma_start(
                            xt_sb[:], xt_d[:, ts].rearrange("(kc p) t -> p kc t", p=P)
                        )
                        qorder = korder = list(range(NKC))
                    for w_sb, dst, ub, brow, ceng, order in (
                        (wq_sb, qt_all, use_bq, 0, nc.vector.tensor_copy, qorder),
                        (wk_sb, kt_all, use_bk, 1, nc.scalar.copy, korder),
                    ):
                        pss = [psA.tile([P, 512], F32, tag="pp", name=f"psqk{x}")
                               for x in range(HPG)]
                        for ki, kc in enumerate(order):
                            for hi in range(HPG):
                                nc.tensor.matmul(
                                    pss[hi][:],
                                    w_sb[:, kc, hi * D:(hi + 1) * D],
                                    xt_sb[:, kc, :],
                                    start=(ki == 0),
                                    stop=(ki == NKC - 1 and not ub),
                                )
                        for hi in range(HPG):
                            if ub:
                                nc.tensor.matmul(
                                    pss[hi][:],
                                    bqkv_sb[brow:brow + 1, hi * D:(hi + 1) * D],
                                    onesrow_sb[:],
                                    start=False,
                                    stop=True,
                                )
                            ceng(dst[:, hi, ts], pss[hi][:])
                    pss = [psA.tile([P, 512], F32, tag="pp", name=f"psv{x}")
                           for x in range(4)]
                    for kc in range(NKC):
                        for tt in range(4):
                            nc.tensor.matmul(
                                pss[tt][:],
                                xt_sb[:, kc, tt * P:(tt + 1) * P],
                                wv_sb[:, kc, :],
                                start=(kc == 0),
                                stop=(kc == NKC - 1 and not use_bv),
                            )
                    for tt in range(4):
                        gt = 4 * tn + tt
                        if use_bv:
                            nc.tensor.matmul(
                                pss[tt][:],
                                onesrow_sb[:, :P],
                                bqkv_sb[2:3, :],
                                start=False,
                                stop=True,
                            )
                        nc.vector.tensor_copy(v_all[:, gt, :], pss[tt][:])

            # wo prefetch: gpsimd queue is free from here; only needed at the
            # first phase-C block, ~10s of us away.
            for h in range(HPG):
                nc.gpsimd.dma_start(wo_sb[:, h, :], wo_d[h * P:(h + 1) * P, :])

            # ---------------- Phase B + C, fused per chunk ----------------
            with (
                tc.tile_pool(name="ep", bufs=2) as ep,
                tc.tile_pool(name="rp", bufs=2) as rp,
                tc.tile_pool(name="stC", bufs=4) as stC,
                tc.tile_pool(name="psX", bufs=4, space="PSUM") as psX,
                tc.tile_pool(name="psO", bufs=2, space="PSUM") as psO,
                tc.tile_pool(name="psD", bufs=2, space="PSUM") as psD,
            ):
                # psX serves both the S tiles (head loops) and the phase-C
                # chains (between head loops) - they never need banks at once.
                psS = psC = psX
                yqueues = [nc.sync, nc.scalar, nc.gpsimd, nc.sync]
                LAG = 3  # tiles of PV/den lag so the PE never waits on exp

                pend = []

                def emit_pending():
                    """Emit the oldest pending PV+den pair; finalize its head
                    when it is the last tile of that head's chunk."""
                    (phi, pj, pidx, pkt, pn, poff, pe_sb, po_ps, pden_ps) = pend.pop(0)
                    nc.tensor.matmul(
                        po_ps[:, poff:],
                        v_all[:, pkt, phi * D:(phi + 1) * D],
                        pe_sb[:, pidx, poff:],
                        start=(pidx == 0),
                        stop=(pidx == pn - 1),
                    )
                    nc.tensor.matmul(
                        pden_ps[:, poff:],
                        ones_sb[:, :P],
                        pe_sb[:, pidx, poff:],
                        start=(pidx == 0),
                        stop=(pidx == pn - 1),
                    )
                    if pidx == pn - 1:
                        rec = rp.tile([P, 512], F32, tag="rec", name="rec")
                        nc.vector.reciprocal_approx_fast(rec[:], pden_ps[:])
                        nc.vector.tensor_mul(
                            on_all[:, phi, pj * 512:(pj + 1) * 512],
                            po_ps[:], rec[:],
                        )

                for j in reversed(range(NQC)):
                    for hi in range(HPG):
                        kts = _kts_for_chunk(hi, j)
                        n = len(kts)
                        e_sb = ep.tile([P, NKT, 512], BF16, tag="e", name="e_sb")
                        o_ps = psO.tile([P, 512], F32, tag="op", name="o_ps")
                        den_ps = psD.tile([P, 512], F32, tag="dp", name="den_ps")
                        use_qbc = hi <= 1
                        for idx, kt in enumerate(kts):
                            # live column range: diagonal tiles start later
                            off = max(0, 128 * kt - 512 * j)
                            s_ps = psS.tile([P, 512], F32, tag="sp", name="s_ps")
                            if use_qbc:
                                # query-side shift preloaded into PSUM by a
                                # rank-1 matmul (softmax-invariant; range only)
                                nc.tensor.matmul(
                                    s_ps[:],
                                    ones_sb[0:1, :P],
                                    qrow_sb[:, hi, j, :],
                                    start=True,
                                    stop=False,
                                )
                            diag = 128 * kt > 512 * j - 128
                            nc.tensor.matmul(
                                s_ps[:, off:],
                                kt_all[:, hi, kt * P:(kt + 1) * P],
                                qt_all[:, hi, j * 512 + off:(j + 1) * 512],
                                start=not use_qbc,
                                stop=not diag,
                            )
                            if diag:
                                # accumulate -1000 on the causal triangle so
                                # exp underflows to exact zero there
                                nc.tensor.matmul(
                                    s_ps[:, off:],
                                    step_sb[:],
                                    wide_sb[:, 384:896 - off],
                                    start=False,
                                    stop=True,
                                )
                            nc.scalar.activation(
                                e_sb[:, idx, off:],
                                s_ps[:, off:],
                                EXP,
                                bias=ktab_sb[:, hi, j, kt:kt + 1],
                                scale=1.0,
                            )
                            while len(pend) > LAG:
                                emit_pending()
                            pend.append((hi, j, idx, kt, n, off, e_sb, o_ps, den_ps))
                    # drain before phase C so the PE stream stays in dep order
                    while pend:
                        emit_pending()
                    # ---- Phase C for this chunk ----
                    for tt in range(4):
                        gt = 4 * j + tt
                        tsl = slice(gt * P, (gt + 1) * P)
                        for cn in range(NQC):
                            ps = psC.tile([P, 512], F32, tag="sp")
                            for hi in range(HPG):
                                nc.tensor.matmul(
                                    ps[:],
                                    on_all[:, hi, tsl],
                                    wo_sb[:, hi, cn * 512:(cn + 1) * 512],
                                    start=(hi == 0),
                                    stop=(hi == HPG - 1),
                                )
                            st = stC.tile([P, 512], BF16, tag="st")
                            if cn % 2:
                                nc.vector.tensor_copy(st[:], ps[:])
                            else:
                                nc.scalar.copy(st[:], ps[:])
                            q = yqueues[cn] if j > 0 else (
                                nc.sync if cn % 2 else nc.scalar)
                            q.dma_start(
                                y_d[tsl, cn * 512:(cn + 1) * 512], st[:]
                            )

    nc.compile()
    _PROG_CACHE[use_b] = nc
    return nc


def _host_inputs(x, Wq, bq, Wk, bk, Wv, bv, Wo, bo, use_b):
    """Build the 8 per-core input maps."""
    x = np.asarray(x, np.float32)
    Wq = np.asarray(Wq, np.float32)
    Wk = np.asarray(Wk, np.float32)
    Wv = np.asarray(Wv, np.float32)
    Wo = np.asarray(Wo, np.float32)
    bq = np.asarray(bq, np.float32)
    bk = np.asarray(bk, np.float32)
    bv = np.asarray(bv, np.float32)

    ones = np.ones((P, 512), BF)
    onesrow = np.ones((1, 512), BF)
    kk = np.arange(P)
    step = (-1000.0 * (kk[:, None] < kk[None, :])).astype(BF)
    wide = np.zeros((P, 896), np.float32)
    wide[kk, kk + 384] = 1.0
    wide = wide.astype(BF)
    in_maps = []
    for c in range(8):
        b, g = divmod(c, 4)
        heads = _core_heads(g)
        cols = np.concatenate([np.arange(h * D, (h + 1) * D) for h in heads])
        xt = np.ascontiguousarray(x[b].T).astype(BF)
        wq = (Wq[:, cols] * np.float32(1.0 / SQD)).astype(BF)
        wk = Wk[:, cols].astype(BF)
        wv = Wv[:, cols].astype(BF)
        wo = np.ascontiguousarray(Wo[cols, :]).astype(BF)

        # ALiBi split: key-side ramp s*(tk-center) is an exact fp32
        # per-partition exp-bias table (ktab); for the steep head positions
        # the query side -s*(tq-1024) is folded in by a rank-1 PSUM preload.
        # Row-constant rounding of qrow cancels in softmax.
        qrow = np.zeros((HPG, NQC, 512), np.float32)
        ktab = np.zeros((P, HPG, NQC, NKT), np.float32)
        p64 = np.arange(P, dtype=np.float64)
        for hi, h in enumerate(heads):
            s = SLOPES[h]
            for j in range(NQC):
                tq = 512.0 * j + np.arange(512, dtype=np.float64)
                qrow[hi, j] = (-s * (tq - 1024.0)).astype(np.float32)
                center = 1024.0 if hi <= 1 else 512.0 * j + 511.0
                for kt in range(NKT):
                    ktab[:, hi, j, kt] = (
                        s * (128.0 * kt + p64 - center)
                    ).astype(np.float32)
        m = {
            "xt": xt, "wq": wq, "wk": wk, "wv": wv, "wo": wo,
            "qrow": qrow.astype(BF).reshape(1, HPG * NQC * 512),
            "ktab": ktab.reshape(P, HPG * NQC * NKT),
            "ones": ones, "step": step, "wide": wide,
        }
        if any(use_b):
            bqkv = np.stack([
                bq[cols] * np.float32(1.0 / SQD), bk[cols], bv[cols]
            ]).astype(BF)
            m["bqkv"] = bqkv
            m["onesrow"] = onesrow
        in_maps.append(m)
    return in_maps


def _gather(results, bo):
    out = np.zeros((B, T, C), np.float32)
    for c in range(8):
        b = c // 4
        out[b] += np.asarray(results[c]["y"], dtype=np.float32)
    out += np.asarray(bo, np.float32)[None, None, :]
    return out


def run(inputs, trace=False, tmpdir=None, trace_cores=None):
    """Full pipeline; returns (output, BassKernelResults)."""
    x = inputs["x"]
    use_b = (
        bool(np.any(inputs["bq"])),
        bool(np.any(inputs["bk"])),
        bool(np.any(inputs["bv"])),
    )
    nc = _build_program(use_b)
    in_maps = _host_inputs(
        x, inputs["Wq"], inputs["bq"], inputs["Wk"], inputs["bk"],
        inputs["Wv"], inputs["bv"], inputs["Wo"], inputs["bo"], use_b
    )
    res = run_bass_kernel_spmd(
        nc, in_maps, list(range(8)), trace=trace, tmpdir=tmpdir,
        trace_cores=trace_cores,
    )
    out = _gather(res.results, inputs["bo"])
    return out, res


def kernel(**inputs):
    out, _ = run(inputs, trace=False)
    return out


# revision 38
# speedup vs baseline: 1.0019x; 1.0019x over previous
"""Trainium2 Bass kernel for nn_CausalAttention (B=2, T=2048, C=2048, H=16, ALiBi).

Sharding: 8 cores = 2 (batch) x 4 (head groups). Core c handles batch c//4 and
heads [g, g+4, g+8, g+12] where g = c%4 (strided so the ALiBi slope mix is
balanced across cores). One SPMD program; every slope-dependent value enters
as data (exp-bias table, query-shift rows), never as a program constant.

All matmul operands are bf16 (fp32 PSUM accumulation): rel err ~3e-3 on the
final output, well inside the gate, and it halves DMA/SBUF and enables the
PE fast-weight-load path. Everything is SBUF-resident; the only HBM traffic
is the inputs (x^T + weights, bf16) and the fp32 partial-output store.

Per-core device pipeline:
  A) qT/kT [d,t] and v [t,d] projections from host-pretransposed x^T, streamed
     by 512-wide t-slices, weights and x slices arriving kc-chunked on four
     parallel DMA queues so the first matmul unblocks within ~1us. Wq is
     host-prescaled by 1/sqrt(D). All of qT/kT/v stays in SBUF (bf16).
  B) Per query chunk j (descending, biggest first), per head: S^T[tk,tq] =
     kT.T @ qT in PSUM. ALiBi enters as (i) an exact fp32 per-partition
     exp-bias column from a host table (key-side ramp; 1024-centred for the
     two steep head positions, chunk-end-centred for the shallow two) and
     (ii) for the steep positions a query-side shift row folded in by a
     rank-1 matmul PSUM preload (softmax-invariant; range control only).
     ACT computes E = exp(.) into SBUF bf16; GPSIMD masks diagonal tiles
     (affine_select, fill 0). PV and the denominator both accumulate on the
     PE (den via an all-ones stationary, output pre-broadcast across
     partitions), so no vector-engine reduction chain exists. DVE only does
     the reciprocal + normalize per (head, chunk). Diagonal tiles compute
     only the live column range. Far tiles with slope*(tq-tk) >= 150
     everywhere are skipped (exp underflows to 0 in the fp32 reference too).
  C) Interleaved per chunk j, right after its 4 heads: out[t,c] partial =
     sum_h O_norm_h^T.T @ Wo_h from SBUF, stores fanned over all four DMA
     queues. Host sums the 4 head-group partials per batch and adds bo.
Key bias bk cancels in softmax; bq/bv (zero in practice) are otherwise
added on-device via K=1 outer-product matmuls.
"""

import math
import sys

sys.path.insert(0, "/opt/trn_rl_repo")

import numpy as np
import ml_dtypes

import concourse.mybir as mybir  # noqa: E402
import concourse.tile as tile  # noqa: E402
from concourse import bacc  # noqa: E402
from concourse.bass_utils import run_bass_kernel_spmd  # noqa: E402

B, T, C, H = 2, 2048, 2048, 16
D = C // H  # 128
P = 128
NKC = C // P       # 16 contraction tiles
NKT = T // P       # 16 key tiles
NQC = T // 512     # 4 query chunks of 512
HPG = 4            # heads per core
SQD = math.sqrt(D)
SKIP_CUT = 40.0  # skipped tiles have softmax weight <= e^-30: far below the gate
F32 = mybir.dt.float32
BF16 = mybir.dt.bfloat16
EXP = mybir.ActivationFunctionType.Exp
BF = ml_dtypes.bfloat16


def _slopes(n=16):
    start = 2.0 ** (-2.0 ** -(math.log2(n) - 3))
    return [start * start**i for i in range(n)]


SLOPES = _slopes(H)


def _core_heads(g):
    return [g, g + 4, g + 8, g + 12]


def _kts_for_chunk(hi, j):
    # Union over cores: the smallest slope in head-position hi is head 4*hi+3.
    s = SLOPES[4 * hi + 3]
    out = []
    for kt in range(4 * j + 4):
        mind = 512 * j - 128 * kt - 127
        if s * mind < SKIP_CUT:
            out.append(kt)
    return out


_PROG_CACHE = {}


def _build_program(use_b):
    if use_b in _PROG_CACHE:
        return _PROG_CACHE[use_b]
    use_bq, use_bk, use_bv = use_b

    nc = bacc.Bacc(None)
    xt_d = nc.declare_dram_parameter("xt", [C, T], BF16, isOutput=False)
    wq_d = nc.declare_dram_parameter("wq", [C, HPG * D], BF16, isOutput=False)
    wk_d = nc.declare_dram_parameter("wk", [C, HPG * D], BF16, isOutput=False)
    wv_d = nc.declare_dram_parameter("wv", [C, HPG * D], BF16, isOutput=False)
    wo_d = nc.declare_dram_parameter("wo", [HPG * D, C], BF16, isOutput=False)
    qrow_d = nc.declare_dram_parameter("qrow", [33, HPG * NQC * 512], BF16, isOutput=False)
    ktab_d = nc.declare_dram_parameter("ktab", [P, HPG * NQC * NKT], F32, isOutput=False)
    ones_d = nc.declare_dram_parameter("ones", [P, 512], BF16, isOutput=False)
    # causal-mask matmul constants: step[k,p] = -1000*[k<p]; wide one-hot
    # wide[k,g] = [g == k+384]. step.T @ wide[:, 384:896-off] adds -1000 on
    # the masked triangle of a diagonal S tile, so exp gives exact zeros.
    step_d = nc.declare_dram_parameter("step", [P, P], BF16, isOutput=False)
    wide_d = nc.declare_dram_parameter("wide", [P, 896], BF16, isOutput=False)
    if any(use_b):
        bqkv_d = nc.declare_dram_parameter("bqkv", [3, HPG * D], BF16, isOutput=False)
        onesrow_d = nc.declare_dram_parameter("onesrow", [1, 512], BF16, isOutput=False)
    y_d = nc.declare_dram_parameter("y", [T, C], BF16, isOutput=True)

    with tile.TileContext(nc) as tc:
        with (
            tc.tile_pool(name="perm", bufs=1) as perm,
            tc.tile_pool(name="dram", bufs=1, space="DRAM") as dpool,
        ):
            ones_sb = perm.tile([P, 512], BF16, tag="ones")
            nc.sync.dma_start(ones_sb[:], ones_d[:])
            step_sb = perm.tile([P, P], BF16, tag="step")
            nc.sync.dma_start(step_sb[:], step_d[:])
            wide_sb = perm.tile([P, 896], BF16, tag="wide")
            nc.sync.dma_start(wide_sb[:], wide_d[:])
            ktab_sb = perm.tile([P, HPG, NQC, NKT], F32, tag="ktab")
            nc.sync.dma_start(
                ktab_sb[:],
                ktab_d[:].rearrange("p (h j k) -> p h j k", h=HPG, j=NQC),
            )
            qrow_sb = perm.tile([33, HPG, NQC, 512], BF16, tag="qrow")
            nc.sync.dma_start(
                qrow_sb[:],
                qrow_d[:].rearrange("o (h j f) -> o h j f", h=HPG, j=NQC),
            )
            if any(use_b):
                bqkv_sb = perm.tile([3, HPG * D], BF16, tag="bqkv")
                onesrow_sb = perm.tile([1, 512], BF16, tag="onesrow")
                nc.sync.dma_start(bqkv_sb[:], bqkv_d[:])
                nc.sync.dma_start(onesrow_sb[:], onesrow_d[:])

            # SBUF-resident projections + attention outputs (bf16).
            qt_all = perm.tile([P, HPG, T], BF16, tag="qt")
            kt_all = perm.tile([P, HPG, T], BF16, tag="kt")
            v_all = perm.tile([P, NKT, HPG * D], BF16, tag="v")
            on_all = perm.tile([P, HPG, T], BF16, tag="on")
            wo_sb = perm.tile([P, HPG, C], BF16, tag="wo")

            # ---------------- Phase A: projections ----------------
            with (
                tc.tile_pool(name="xtp", bufs=2) as xtp,
                tc.tile_pool(name="wp", bufs=1) as wp,
                tc.tile_pool(name="psA", bufs=8, space="PSUM") as psA,
            ):
                wq_sb = wp.tile([P, NKC, HPG * D], BF16, tag="wq")
                wk_sb = wp.tile([P, NKC, HPG * D], BF16, tag="wk")
                wv_sb = wp.tile([P, NKC, HPG * D], BF16, tag="wv")
                # The scalar+sync queues share one HWDGE ring set (~190GB/s),
                # gpsimd drives SWDGE (~150GB/s); ring order is issue order.
                # Interleave wq/xt(tn0) chunk pairs so the first Q chain's
                # inputs land together, splitting the tail onto SWDGE; then
                # queue the rest on SWDGE in need-order (wk, wv, xt1-3, wo).
                xt0_sb = xtp.tile([P, NKC, 512], BF16, tag="xt")
                for kc in range(NKC):
                    eng = nc.scalar if kc < 10 else nc.gpsimd
                    eng.dma_start(wq_sb[:, kc, :], wq_d[kc * P:(kc + 1) * P, :])
                    eng.dma_start(xt0_sb[:, kc, :], xt_d[kc * P:(kc + 1) * P, 0:512])
                for kc in range(NKC):
                    # low kc on the HWDGE stream right behind the pairs; high
                    # kc leads SWDGE so the K accumulation chains never wait
                    eng = nc.scalar if kc < 8 else nc.gpsimd
                    eng.dma_start(wk_sb[:, kc, :], wk_d[kc * P:(kc + 1) * P, :])
                for kc in range(NKC):
                    nc.gpsimd.dma_start(
                        wv_sb[:, kc, :], wv_d[kc * P:(kc + 1) * P, :]
                    )

                # PE warm-up across the initial DMA window (HAM reaches
                # K=8/8 before the projection chains start), doubling as a
                # microbench: 16 N=512 then 16 N=1024 bf16 matmuls.
                wb_ps = psA.tile([P, 512], F32, tag="pp")
                for wi in range(24):
                    nc.tensor.matmul(
                        wb_ps[:], ones_sb[:, :P], ones_sb[:],
                        start=True, stop=True,
                    )
                warm_out = wp.tile([P, 512], F32, tag="wout")
                nc.vector.tensor_copy(warm_out[:], wb_ps[:])
                warm_d = dpool.tile([P, 512], F32, tag="warmd", name="warm_d")
                nc.sync.dma_start(warm_d[:], warm_out[:])

                for tn in range(NQC):
                    ts = slice(tn * 512, (tn + 1) * 512)
                    if tn == 0:
                        xt_sb = xt0_sb
                        # consume kc in DMA arrival order: SWDGE chunks land
                        # first, then the HWDGE stream, so the four
                        # interleaved chains chase the transfers without gaps
                        qorder = list(range(10, NKC)) + list(range(10))
                        korder = list(range(8, NKC)) + list(range(8))
                    else:
                        xt_sb = xtp.tile([P, NKC, 512], BF16, tag="xt")
                        nc.gpsimd.dma_start(
                            xt_sb[:], xt_d[:, ts].rearrange("(kc p) t -> p kc t", p=P)
                        )
                        qorder = korder = list(range(NKC))
                    def qk_section(w_sb, dst, ub, brow, ceng, order):
                        pss = [psA.tile([P, 512], F32, tag="pp", name=f"psqk{x}")
                               for x in range(HPG)]
                        for ki, kc in enumerate(order):
                            for hi in range(HPG):
                                nc.tensor.matmul(
                                    pss[hi][:],
                                    w_sb[:, kc, hi * D:(hi + 1) * D],
                                    xt_sb[:, kc, :],
                                    start=(ki == 0),
                                    stop=(ki == NKC - 1 and not ub),
                                )
                        for hi in range(HPG):
                            if ub:
                                nc.tensor.matmul(
                                    pss[hi][:],
                                    bqkv_sb[brow:brow + 1, hi * D:(hi + 1) * D],
                                    onesrow_sb[:],
                                    start=False,
                                    stop=True,
                                )
                            ceng(dst[:, hi, ts], pss[hi][:])

                    def v_section():
                        pss = [psA.tile([P, 512], F32, tag="pp", name=f"psv{x}")
                               for x in range(4)]
                        for kc in range(NKC):
                            for tt in range(4):
                                nc.tensor.matmul(
                                    pss[tt][:],
                                    xt_sb[:, kc, tt * P:(tt + 1) * P],
                                    wv_sb[:, kc, :],
                                    start=(kc == 0),
                                    stop=(kc == NKC - 1 and not use_bv),
                                )
                        for tt in range(4):
                            gt = 4 * tn + tt
                            if use_bv:
                                nc.tensor.matmul(
                                    pss[tt][:],
                                    onesrow_sb[:, :P],
                                    bqkv_sb[2:3, :],
                                    start=False,
                                    stop=True,
                                )
                            nc.vector.tensor_copy(v_all[:, gt, :], pss[tt][:])

                    # tn0 must run [Q,K,V] (wv arrives last on SWDGE); later
                    # tns run [V,K,Q] so the A->B PSUM-bank handoff waits only
                    # on the short Q-copy tail, not the V-copy tail.
                    if tn == 0:
                        qk_section(wq_sb, qt_all, use_bq, 0,
                                   nc.vector.tensor_copy, qorder)
                        qk_section(wk_sb, kt_all, use_bk, 1,
                                   nc.scalar.copy, korder)
                        v_section()
                    else:
                        v_section()
                        qk_section(wk_sb, kt_all, use_bk, 1,
                                   nc.scalar.copy, korder)
                        qk_section(wq_sb, qt_all, use_bq, 0,
                                   nc.vector.tensor_copy, qorder)

            # wo prefetch: gpsimd queue is free from here; only needed at the
            # first phase-C block, ~10s of us away.
            for h in range(HPG):
                nc.gpsimd.dma_start(wo_sb[:, h, :], wo_d[h * P:(h + 1) * P, :])

            # ---------------- Phase B + C, fused per chunk ----------------
            with (
                tc.tile_pool(name="ep", bufs=2) as ep,
                tc.tile_pool(name="rp", bufs=2) as rp,
                tc.tile_pool(name="stC", bufs=4) as stC,
                tc.tile_pool(name="psX", bufs=4, space="PSUM") as psX,
                tc.tile_pool(name="psO", bufs=2, space="PSUM") as psO,
                tc.tile_pool(name="psD", bufs=2, space="PSUM") as psD,
            ):
                # psX serves both the S tiles (head loops) and the phase-C
                # chains (between head loops) - they never need banks at once.
                psS = psC = psX
                yqueues = [nc.sync, nc.scalar, nc.gpsimd, nc.sync]
                LAG = 3  # tiles of PV/den lag so the PE never waits on exp

                pend = []

                def emit_pending():
                    """Emit the oldest pending PV+den pair; finalize its head
                    when it is the last tile of that head's chunk."""
                    (phi, pj, pidx, pkt, pn, poff, pe_sb, po_ps, pden_ps) = pend.pop(0)
                    nc.tensor.matmul(
                        po_ps[:, poff:],
                        v_all[:, pkt, phi * D:(phi + 1) * D],
                        pe_sb[:, pidx, poff:],
                        start=(pidx == 0),
                        stop=(pidx == pn - 1),
                    )
                    nc.tensor.matmul(
                        pden_ps[:, poff:],
                        ones_sb[:, :P],
                        pe_sb[:, pidx, poff:],
                        start=(pidx == 0),
                        stop=(pidx == pn - 1),
                    )
                    if pidx == pn - 1:
                        rec = rp.tile([P, 512], F32, tag="rec", name="rec")
                        nc.vector.reciprocal_approx_fast(rec[:], pden_ps[:])
                        nc.vector.tensor_mul(
                            on_all[:, phi, pj * 512:(pj + 1) * 512],
                            po_ps[:], rec[:],
                        )

                for j in reversed(range(NQC)):
                    for hi in range(HPG):
                        kts = _kts_for_chunk(hi, j)
                        n = len(kts)
                        e_sb = ep.tile([P, NKT, 512], BF16, tag="e", name="e_sb")
                        o_ps = psO.tile([P, 512], F32, tag="op", name="o_ps")
                        den_ps = psD.tile([P, 512], F32, tag="dp", name="den_ps")
                        use_qbc = hi <= 1
                        s_ps_next = None
                        for idx, kt in enumerate(kts):
                            # live column range: diagonal tiles start later
                            off = max(0, 128 * kt - 512 * j)
                            if use_qbc:
                                # query-side shift preloaded into PSUM by a
                                # rank-1 matmul (softmax-invariant; range
                                # only); consecutive tiles share one packed
                                # PE pass via distinct row groups
                                if s_ps_next is not None:
                                    s_ps = s_ps_next
                                    s_ps_next = None
                                else:
                                    s_ps = psS.tile([P, 512], F32, tag="sp",
                                                    name="s_ps")
                                    nc.tensor.matmul(
                                        s_ps[:],
                                        ones_sb[0:1, :P],
                                        qrow_sb[0:1, hi, j, :],
                                        start=True,
                                        stop=False,
                                    )
                                    if idx + 1 < n:
                                        s_ps_next = psS.tile(
                                            [P, 512], F32, tag="sp",
                                            name="s_ps_n")
                                        nc.tensor.matmul(
                                            s_ps_next[:],
                                            ones_sb[32:33, :P],
                                            qrow_sb[32:33, hi, j, :],
                                            start=True,
                                            stop=False,
                                            tile_position=(32, 0),
                                        )
                            else:
                                s_ps = psS.tile([P, 512], F32, tag="sp",
                                                name="s_ps")
                            diag = 128 * kt > 512 * j - 128
                            nc.tensor.matmul(
                                s_ps[:, off:],
                                kt_all[:, hi, kt * P:(kt + 1) * P],
                                qt_all[:, hi, j * 512 + off:(j + 1) * 512],
                                start=not use_qbc,
                                stop=not diag,
                            )
                            if diag:
                                # accumulate -1000 on the causal triangle so
                                # exp underflows to exact zero there
                                nc.tensor.matmul(
                                    s_ps[:, off:],
                                    step_sb[:],
                                    wide_sb[:, 384:896 - off],
                                    start=False,
                                    stop=True,
                                )
                            nc.scalar.activation(
                                e_sb[:, idx, off:],
                                s_ps[:, off:],
                                EXP,
                                bias=ktab_sb[:, hi, j, kt:kt + 1],
                                scale=1.0,
                            )
                            while len(pend) > LAG:
                                emit_pending()
                            pend.append((hi, j, idx, kt, n, off, e_sb, o_ps, den_ps))
                    # drain before phase C so the PE stream stays in dep order
                    while pend:
                        emit_pending()
                    # ---- Phase C for this chunk ----
                    for tt in range(4):
                        gt = 4 * j + tt
                        tsl = slice(gt * P, (gt + 1) * P)
                        for cn in range(NQC):
                            ps = psC.tile([P, 512], F32, tag="sp")
                            for hi in range(HPG):
                                nc.tensor.matmul(
                                    ps[:],
                                    on_all[:, hi, tsl],
                                    wo_sb[:, hi, cn * 512:(cn + 1) * 512],
                                    start=(hi == 0),
                                    stop=(hi == HPG - 1),
                                )
                            st = stC.tile([P, 512], BF16, tag="st")
                            if cn % 2:
                                nc.vector.tensor_copy(st[:], ps[:])
                            else:
                                nc.scalar.copy(st[:], ps[:])
                            q = yqueues[cn] if j > 0 else (
                                nc.sync if cn % 2 else nc.scalar)
                            q.dma_start(
                                y_d[tsl, cn * 512:(cn + 1) * 512], st[:]
                            )

    nc.compile()
    _PROG_CACHE[use_b] = nc
    return nc


def _host_inputs(x, Wq, bq, Wk, bk, Wv, bv, Wo, bo, use_b):
    """Build the 8 per-core input maps."""
    x = np.asarray(x, np.float32)
    Wq = np.asarray(Wq, np.float32)
    Wk = np.asarray(Wk, np.float32)
    Wv = np.asarray(Wv, np.float32)
    Wo = np.asarray(Wo, np.float32)
    bq = np.asarray(bq, np.float32)
    bk = np.asarray(bk, np.float32)
    bv = np.asarray(bv, np.float32)

    ones = np.ones((P, 512), BF)
    onesrow = np.ones((1, 512), BF)
    kk = np.arange(P)
    step = (-1000.0 * (kk[:, None] < kk[None, :])).astype(BF)
    wide = np.zeros((P, 896), np.float32)
    wide[kk, kk + 384] = 1.0
    wide = wide.astype(BF)
    in_maps = []
    for c in range(8):
        b, g = divmod(c, 4)
        heads = _core_heads(g)
        cols = np.concatenate([np.arange(h * D, (h + 1) * D) for h in heads])
        xt = np.ascontiguousarray(x[b].T).astype(BF)
        wq = (Wq[:, cols] * np.float32(1.0 / SQD)).astype(BF)
        wk = Wk[:, cols].astype(BF)
        wv = Wv[:, cols].astype(BF)
        wo = np.ascontiguousarray(Wo[cols, :]).astype(BF)

        # ALiBi split: key-side ramp s*(tk-center) is an exact fp32
        # per-partition exp-bias table (ktab); for the steep head positions
        # the query side -s*(tq-1024) is folded in by a rank-1 PSUM preload.
        # Row-constant rounding of qrow cancels in softmax.
        qrow = np.zeros((HPG, NQC, 512), np.float32)
        ktab = np.zeros((P, HPG, NQC, NKT), np.float32)
        p64 = np.arange(P, dtype=np.float64)
        for hi, h in enumerate(heads):
            s = SLOPES[h]
            for j in range(NQC):
                tq = 512.0 * j + np.arange(512, dtype=np.float64)
                qrow[hi, j] = (-s * (tq - 1024.0)).astype(np.float32)
                center = 1024.0 if hi <= 1 else 512.0 * j + 511.0
                for kt in range(NKT):
                    ktab[:, hi, j, kt] = (
                        s * (128.0 * kt + p64 - center)
                    ).astype(np.float32)
        qrow2 = np.zeros((33, HPG * NQC * 512), np.float32)
        qrow2[0] = qrow2[32] = qrow.reshape(-1)
        m = {
            "xt": xt, "wq": wq, "wk": wk, "wv": wv, "wo": wo,
            "qrow": qrow2.astype(BF),
            "ktab": ktab.reshape(P, HPG * NQC * NKT),
            "ones": ones, "step": step, "wide": wide,
        }
        if any(use_b):
            bqkv = np.stack([
                bq[cols] * np.float32(1.0 / SQD), bk[cols], bv[cols]
            ]).astype(BF)
            m["bqkv"] = bqkv
            m["onesrow"] = onesrow
        in_maps.append(m)
    return in_maps


def _gather(results, bo):
    out = np.zeros((B, T, C), np.float32)
    for c in range(8):
        b = c // 4
        out[b] += np.asarray(results[c]["y"], dtype=np.float32)
    out += np.asarray(bo, np.float32)[None, None, :]
    return out


def run(inputs, trace=False, tmpdir=None, trace_cores=None):
    """Full pipeline; returns (output, BassKernelResults)."""
    x = inputs["x"]
    use_b = (
        bool(np.any(inputs["bq"])),
        bool(np.any(inputs["bk"])),
        bool(np.any(inputs["bv"])),
    )
    nc = _build_program(use_b)
    in_maps = _host_inputs(
        x, inputs["Wq"], inputs["bq"], inputs["Wk"], inputs["bk"],
        inputs["Wv"], inputs["bv"], inputs["Wo"], inputs["bo"], use_b
    )
    res = run_bass_kernel_spmd(
        nc, in_maps, list(range(8)), trace=trace, tmpdir=tmpdir,
        trace_cores=trace_cores,
    )
    out = _gather(res.results, inputs["bo"])
    return out, res


def kernel(**inputs):
    out, _ = run(inputs, trace=False)
    return out


# revision 39
# speedup vs baseline: 1.0563x; 1.0542x over previous
"""Trainium2 Bass kernel for nn_CausalAttention (B=2, T=2048, C=2048, H=16, ALiBi).

Sharding: 8 cores = 2 (batch) x 4 (head groups). Core c handles batch c//4 and
heads [g, g+4, g+8, g+12] where g = c%4 (strided so the ALiBi slope mix is
balanced across cores). One SPMD program; every slope-dependent value enters
as data (exp-bias table, query-shift rows), never as a program constant.

All matmul operands are bf16 (fp32 PSUM accumulation): rel err ~4e-3 on the
final output, well inside the gate; it halves DMA/SBUF and enables the PE
fast-weight-load path. Everything is SBUF-resident - the only HBM traffic is
the inputs (x^T + weights, bf16) and the bf16 partial-output store.

Per-core device pipeline:
  A) qT/kT [d,t] and v [t,d] projections from host-pretransposed x^T by
     512-wide t-slices. DMA rings are issue-ordered: wq/xt(tn0) chunk pairs
     interleave on the HWDGE stream (tail on SWDGE), wk splits across both,
     and the four per-head accumulation chains consume kc chunks in arrival
     order, so the PE chases the transfers without idling (a short warm-up
     block on resident ones keeps the HAM clock-gate at K=8/8). Wq is
     host-prescaled by 1/sqrt(D). tn>=1 sections run [V,K,Q] so the A->B
     PSUM-bank handoff waits only on the short Q-copy tail.
  B) Per query chunk j (descending), per head: S^T[tk,tq] = kT.T @ qT in
     PSUM. ALiBi enters as (i) an exact fp32 per-partition exp-bias column
     (key-side ramp; 1024-centred for the two steep head positions,
     chunk-end-centred for the shallow two) and (ii) for the steep positions
     a query-side shift row folded in by a rank-1 matmul PSUM preload
     (softmax-invariant; range control only), packed two-at-a-time into
     distinct PE row groups via tile_position. Diagonal tiles get -1000
     accumulated on the causal triangle by a constant step x one-hot matmul,
     so ACT's exp (into SBUF bf16) yields exact zeros there - no separate
     masking pass and no cross-engine hop. PV and the denominator (all-ones
     stationary, output pre-broadcast across partitions) accumulate on the
     PE, emitted with a 3-tile software lag so the in-order PE stream never
     waits on exp. DVE only does the reciprocal + normalize per (head,
     chunk). Tiles compute only the live column range; far tiles with
     slope*(tq-tk) >= 40 everywhere are skipped (softmax weight <= e^-30 in
     the fp32 reference as well).
  C) Per chunk j, right after its 4 heads: out[t,c] partial =
     sum_h O_norm_h^T.T @ Wo_h from SBUF, stores fanned over the three DMA
     queues (HWDGE-only for the final chunk to shorten the drain tail).
Host sums the 4 head-group partials per batch and adds bo. Key bias bk
cancels in softmax; bq/bv (zero in practice) are otherwise added on-device
via K=1 outer-product matmuls.
"""

import math
import sys

sys.path.insert(0, "/opt/trn_rl_repo")

import numpy as np
import ml_dtypes

import concourse.mybir as mybir  # noqa: E402
import concourse.tile as tile  # noqa: E402
from concourse import bacc  # noqa: E402
from concourse.bass_utils import run_bass_kernel_spmd  # noqa: E402

B, T, C, H = 2, 2048, 2048, 16
D = C // H  # 128
P = 128
NKC = C // P       # 16 contraction tiles
NKT = T // P       # 16 key tiles
NQC = T // 512     # 4 query chunks of 512
HPG = 4            # heads per core
SQD = math.sqrt(D)
SKIP_CUT = 40.0  # skipped tiles have softmax weight <= e^-30: far below the gate
F32 = mybir.dt.float32
BF16 = mybir.dt.bfloat16
EXP = mybir.ActivationFunctionType.Exp
BF = ml_dtypes.bfloat16


def _slopes(n=16):
    start = 2.0 ** (-2.0 ** -(math.log2(n) - 3))
    return [start * start**i for i in range(n)]


SLOPES = _slopes(H)


def _core_heads(g):
    return [g, g + 4, g + 8, g + 12]


def _kts_for_chunk(hi, j):
    # Union over cores: the smallest slope in head-position hi is head 4*hi+3.
    s = SLOPES[4 * hi + 3]
    out = []
    for kt in range(4 * j + 4):
        mind = 512 * j - 128 * kt - 127
        if s * mind < SKIP_CUT:
            out.append(kt)
    return out


_PROG_CACHE = {}


def _build_program(use_b):
    if use_b in _PROG_CACHE:
        return _PROG_CACHE[use_b]
    use_bq, use_bk, use_bv = use_b

    nc = bacc.Bacc(None)
    xt_d = nc.declare_dram_parameter("xt", [C, T], BF16, isOutput=False)
    wq_d = nc.declare_dram_parameter("wq", [C, HPG * D], BF16, isOutput=False)
    wk_d = nc.declare_dram_parameter("wk", [C, HPG * D], BF16, isOutput=False)
    wv_d = nc.declare_dram_parameter("wv", [C, HPG * D], BF16, isOutput=False)
    wo_d = nc.declare_dram_parameter("wo", [HPG * D, C], BF16, isOutput=False)
    qrow_d = nc.declare_dram_parameter("qrow", [33, HPG * NQC * 512], BF16, isOutput=False)
    ktab_d = nc.declare_dram_parameter("ktab", [P, HPG * NQC * NKT], F32, isOutput=False)
    ones_d = nc.declare_dram_parameter("ones", [P, 512], BF16, isOutput=False)
    # causal-mask matmul constants: step[k,p] = -1000*[k<p]; wide one-hot
    # wide[k,g] = [g == k+384]. step.T @ wide[:, 384:896-off] adds -1000 on
    # the masked triangle of a diagonal S tile, so exp gives exact zeros.
    step_d = nc.declare_dram_parameter("step", [P, P], BF16, isOutput=False)
    wide_d = nc.declare_dram_parameter("wide", [P, 896], BF16, isOutput=False)
    if any(use_b):
        bqkv_d = nc.declare_dram_parameter("bqkv", [3, HPG * D], BF16, isOutput=False)
        onesrow_d = nc.declare_dram_parameter("onesrow", [1, 512], BF16, isOutput=False)
    y_d = nc.declare_dram_parameter("y", [T, C], BF16, isOutput=True)

    with tile.TileContext(nc) as tc:
        with (
            tc.tile_pool(name="perm", bufs=1) as perm,
            tc.tile_pool(name="dram", bufs=1, space="DRAM") as dpool,
        ):
            ones_sb = perm.tile([P, 512], BF16, tag="ones")
            nc.sync.dma_start(ones_sb[:], ones_d[:])
            step_sb = perm.tile([P, P], BF16, tag="step")
            nc.sync.dma_start(step_sb[:], step_d[:])
            wide_sb = perm.tile([P, 896], BF16, tag="wide")
            nc.sync.dma_start(wide_sb[:], wide_d[:])
            ktab_sb = perm.tile([P, HPG, NQC, NKT], F32, tag="ktab")
            nc.sync.dma_start(
                ktab_sb[:],
                ktab_d[:].rearrange("p (h j k) -> p h j k", h=HPG, j=NQC),
            )
            qrow_sb = perm.tile([33, HPG, NQC, 512], BF16, tag="qrow")
            nc.sync.dma_start(
                qrow_sb[:],
                qrow_d[:].rearrange("o (h j f) -> o h j f", h=HPG, j=NQC),
            )
            if any(use_b):
                bqkv_sb = perm.tile([3, HPG * D], BF16, tag="bqkv")
                onesrow_sb = perm.tile([1, 512], BF16, tag="onesrow")
                nc.sync.dma_start(bqkv_sb[:], bqkv_d[:])
                nc.sync.dma_start(onesrow_sb[:], onesrow_d[:])

            # SBUF-resident projections + attention outputs (bf16).
            qt_all = perm.tile([P, HPG, T], BF16, tag="qt")
            kt_all = perm.tile([P, HPG, T], BF16, tag="kt")
            v_all = perm.tile([P, NKT, HPG * D], BF16, tag="v")
            on_all = perm.tile([P, HPG, T], BF16, tag="on")
            wo_sb = perm.tile([P, HPG, C], BF16, tag="wo")

            # ---------------- Phase A: projections ----------------
            with (
                tc.tile_pool(name="xtp", bufs=2) as xtp,
                tc.tile_pool(name="wp", bufs=1) as wp,
                tc.tile_pool(name="psA", bufs=8, space="PSUM") as psA,
            ):
                wq_sb = wp.tile([P, NKC, HPG * D], BF16, tag="wq")
                wk_sb = wp.tile([P, NKC, HPG * D], BF16, tag="wk")
                wv_sb = wp.tile([P, NKC, HPG * D], BF16, tag="wv")
                # The scalar+sync queues share one HWDGE ring set (~190GB/s),
                # gpsimd drives SWDGE (~150GB/s); ring order is issue order.
                # Interleave wq/xt(tn0) chunk pairs so the first Q chain's
                # inputs land together, splitting the tail onto SWDGE; then
                # queue the rest on SWDGE in need-order (wk, wv, xt1-3, wo).
                xt0_sb = xtp.tile([P, NKC, 512], BF16, tag="xt")
                for kc in range(NKC):
                    eng = nc.scalar if kc < 10 else nc.gpsimd
                    eng.dma_start(wq_sb[:, kc, :], wq_d[kc * P:(kc + 1) * P, :])
                    eng.dma_start(xt0_sb[:, kc, :], xt_d[kc * P:(kc + 1) * P, 0:512])
                for kc in range(NKC):
                    # low kc on the HWDGE stream right behind the pairs; high
                    # kc leads SWDGE so the K accumulation chains never wait
                    eng = nc.scalar if kc < 8 else nc.gpsimd
                    eng.dma_start(wk_sb[:, kc, :], wk_d[kc * P:(kc + 1) * P, :])
                for kc in range(NKC):
                    nc.gpsimd.dma_start(
                        wv_sb[:, kc, :], wv_d[kc * P:(kc + 1) * P, :]
                    )

                # PE warm-up across the initial DMA window (HAM reaches
                # K=8/8 before the projection chains start), doubling as a
                # microbench: 16 N=512 then 16 N=1024 bf16 matmuls.
                wb_ps = psA.tile([P, 512], F32, tag="pp")
                for wi in range(24):
                    nc.tensor.matmul(
                        wb_ps[:], ones_sb[:, :P], ones_sb[:],
                        start=True, stop=True,
                    )
                warm_out = wp.tile([P, 512], F32, tag="wout")
                nc.vector.tensor_copy(warm_out[:], wb_ps[:])
                warm_d = dpool.tile([P, 512], F32, tag="warmd", name="warm_d")
                nc.sync.dma_start(warm_d[:], warm_out[:])

                for tn in range(NQC):
                    ts = slice(tn * 512, (tn + 1) * 512)
                    if tn == 0:
                        xt_sb = xt0_sb
                        # consume kc in DMA arrival order: SWDGE chunks land
                        # first, then the HWDGE stream, so the four
                        # interleaved chains chase the transfers without gaps
                        qorder = list(range(10, NKC)) + list(range(10))
                        korder = list(range(8, NKC)) + list(range(8))
                    else:
                        xt_sb = xtp.tile([P, NKC, 512], BF16, tag="xt")
                        nc.gpsimd.dma_start(
                            xt_sb[:], xt_d[:, ts].rearrange("(kc p) t -> p kc t", p=P)
                        )
                        qorder = korder = list(range(NKC))
                    def qk_section(w_sb, dst, ub, brow, ceng, order):
                        pss = [psA.tile([P, 512], F32, tag="pp", name=f"psqk{x}")
                               for x in range(HPG)]
                        for ki, kc in enumerate(order):
                            for hi in range(HPG):
                                nc.tensor.matmul(
                                    pss[hi][:],
                                    w_sb[:, kc, hi * D:(hi + 1) * D],
                                    xt_sb[:, kc, :],
                                    start=(ki == 0),
                                    stop=(ki == NKC - 1 and not ub),
                                )
                        for hi in range(HPG):
                            if ub:
                                nc.tensor.matmul(
                                    pss[hi][:],
                                    bqkv_sb[brow:brow + 1, hi * D:(hi + 1) * D],
                                    onesrow_sb[:],
                                    start=False,
                                    stop=True,
                                )
                            ceng(dst[:, hi, ts], pss[hi][:])

                    def v_section():
                        pss = [psA.tile([P, 512], F32, tag="pp", name=f"psv{x}")
                               for x in range(4)]
                        for kc in range(NKC):
                            for tt in range(4):
                                nc.tensor.matmul(
                                    pss[tt][:],
                                    xt_sb[:, kc, tt * P:(tt + 1) * P],
                                    wv_sb[:, kc, :],
                                    start=(kc == 0),
                                    stop=(kc == NKC - 1 and not use_bv),
                                )
                        for tt in range(4):
                            gt = 4 * tn + tt
                            if use_bv:
                                nc.tensor.matmul(
                                    pss[tt][:],
                                    onesrow_sb[:, :P],
                                    bqkv_sb[2:3, :],
                                    start=False,
                                    stop=True,
                                )
                            nc.vector.tensor_copy(v_all[:, gt, :], pss[tt][:])

                    # tn0 must run [Q,K,V] (wv arrives last on SWDGE); later
                    # tns run [V,K,Q] so the A->B PSUM-bank handoff waits only
                    # on the short Q-copy tail, not the V-copy tail.
                    if tn == 0:
                        qk_section(wq_sb, qt_all, use_bq, 0,
                                   nc.vector.tensor_copy, qorder)
                        qk_section(wk_sb, kt_all, use_bk, 1,
                                   nc.scalar.copy, korder)
                        v_section()
                    else:
                        v_section()
                        qk_section(wk_sb, kt_all, use_bk, 1,
                                   nc.scalar.copy, korder)
                        qk_section(wq_sb, qt_all, use_bq, 0,
                                   nc.vector.tensor_copy, qorder)

            # wo prefetch: gpsimd queue is free from here; only needed at the
            # first phase-C block, ~10s of us away.
            for h in range(HPG):
                nc.gpsimd.dma_start(wo_sb[:, h, :], wo_d[h * P:(h + 1) * P, :])

            # ---------------- Phase B + C, fused per chunk ----------------
            with (
                tc.tile_pool(name="ep", bufs=2) as ep,
                tc.tile_pool(name="rp", bufs=2) as rp,
                tc.tile_pool(name="stC", bufs=4) as stC,
                tc.tile_pool(name="psX", bufs=4, space="PSUM") as psX,
                tc.tile_pool(name="psO", bufs=2, space="PSUM") as psO,
                tc.tile_pool(name="psD", bufs=2, space="PSUM") as psD,
            ):
                # psX serves both the S tiles (head loops) and the phase-C
                # chains (between head loops) - they never need banks at once.
                psS = psC = psX
                yqueues = [nc.sync, nc.scalar, nc.gpsimd, nc.sync]
                LAG = 3  # tiles of PV/den lag so the PE never waits on exp

                pend = []

                def emit_pending():
                    """Emit the oldest pending PV+den pair; finalize its head
                    when it is the last tile of that head's chunk."""
                    (phi, pj, pidx, pkt, pn, poff, pe_sb, po_ps, pden_ps) = pend.pop(0)
                    nc.tensor.matmul(
                        po_ps[:, poff:],
                        v_all[:, pkt, phi * D:(phi + 1) * D],
                        pe_sb[:, pidx, poff:],
                        start=(pidx == 0),
                        stop=(pidx == pn - 1),
                    )
                    nc.tensor.matmul(
                        pden_ps[:, poff:],
                        ones_sb[:, :P],
                        pe_sb[:, pidx, poff:],
                        start=(pidx == 0),
                        stop=(pidx == pn - 1),
                    )
                    if pidx == pn - 1:
                        rec = rp.tile([P, 512], F32, tag="rec", name="rec")
                        nc.vector.reciprocal_approx_fast(rec[:], pden_ps[:])
                        nc.vector.tensor_mul(
                            on_all[:, phi, pj * 512:(pj + 1) * 512],
                            po_ps[:], rec[:],
                        )

                for j in reversed(range(NQC)):
                    for hi in range(HPG):
                        kts = _kts_for_chunk(hi, j)
                        n = len(kts)
                        e_sb = ep.tile([P, NKT, 512], BF16, tag="e", name="e_sb")
                        o_ps = psO.tile([P, 512], F32, tag="op", name="o_ps")
                        den_ps = psD.tile([P, 512], F32, tag="dp", name="den_ps")
                        use_qbc = hi <= 1
                        s_ps_next = None
                        for idx, kt in enumerate(kts):
                            # live column range: diagonal tiles start later
                            off = max(0, 128 * kt - 512 * j)
                            if use_qbc:
                                # query-side shift preloaded into PSUM by a
                                # rank-1 matmul (softmax-invariant; range
                                # only); consecutive tiles share one packed
                                # PE pass via distinct row groups
                                if s_ps_next is not None:
                                    s_ps = s_ps_next
                                    s_ps_next = None
                                else:
                                    s_ps = psS.tile([P, 512], F32, tag="sp",
                                                    name="s_ps")
                                    nc.tensor.matmul(
                                        s_ps[:],
                                        ones_sb[0:1, :P],
                                        qrow_sb[0:1, hi, j, :],
                                        start=True,
                                        stop=False,
                                    )
                                    if idx + 1 < n:
                                        s_ps_next = psS.tile(
                                            [P, 512], F32, tag="sp",
                                            name="s_ps_n")
                                        nc.tensor.matmul(
                                            s_ps_next[:],
                                            ones_sb[32:33, :P],
                                            qrow_sb[32:33, hi, j, :],
                                            start=True,
                                            stop=False,
                                            tile_position=(32, 0),
                                        )
                            else:
                                s_ps = psS.tile([P, 512], F32, tag="sp",
                                                name="s_ps")
                            diag = 128 * kt > 512 * j - 128
                            nc.tensor.matmul(
                                s_ps[:, off:],
                                kt_all[:, hi, kt * P:(kt + 1) * P],
                                qt_all[:, hi, j * 512 + off:(j + 1) * 512],
                                start=not use_qbc,
                                stop=not diag,
                            )
                            if diag:
                                # accumulate -1000 on the causal triangle so
                                # exp underflows to exact zero there
                                nc.tensor.matmul(
                                    s_ps[:, off:],
                                    step_sb[:],
                                    wide_sb[:, 384:896 - off],
                                    start=False,
                                    stop=True,
                                )
                            nc.scalar.activation(
                                e_sb[:, idx, off:],
                                s_ps[:, off:],
                                EXP,
                                bias=ktab_sb[:, hi, j, kt:kt + 1],
                                scale=1.0,
                            )
                            while len(pend) > LAG:
                                emit_pending()
                            pend.append((hi, j, idx, kt, n, off, e_sb, o_ps, den_ps))
                    # drain before phase C so the PE stream stays in dep order
                    while pend:
                        emit_pending()
                    # ---- Phase C for this chunk ----
                    for tt in range(4):
                        gt = 4 * j + tt
                        tsl = slice(gt * P, (gt + 1) * P)
                        for cn in range(NQC):
                            ps = psC.tile([P, 512], F32, tag="sp")
                            for hi in range(HPG):
                                nc.tensor.matmul(
                                    ps[:],
                                    on_all[:, hi, tsl],
                                    wo_sb[:, hi, cn * 512:(cn + 1) * 512],
                                    start=(hi == 0),
                                    stop=(hi == HPG - 1),
                                )
                            st = stC.tile([P, 512], BF16, tag="st")
                            if cn % 2:
                                nc.vector.tensor_copy(st[:], ps[:])
                            else:
                                nc.scalar.copy(st[:], ps[:])
                            q = yqueues[cn] if j > 0 else (
                                nc.sync if cn % 2 else nc.scalar)
                            q.dma_start(
                                y_d[tsl, cn * 512:(cn + 1) * 512], st[:]
                            )

    nc.compile()
    _PROG_CACHE[use_b] = nc
    return nc


def _host_inputs(x, Wq, bq, Wk, bk, Wv, bv, Wo, bo, use_b):
    """Build the 8 per-core input maps."""
    x = np.asarray(x, np.float32)
    Wq = np.asarray(Wq, np.float32)
    Wk = np.asarray(Wk, np.float32)
    Wv = np.asarray(Wv, np.float32)
    Wo = np.asarray(Wo, np.float32)
    bq = np.asarray(bq, np.float32)
    bk = np.asarray(bk, np.float32)
    bv = np.asarray(bv, np.float32)

    ones = np.ones((P, 512), BF)
    onesrow = np.ones((1, 512), BF)
    kk = np.arange(P)
    step = (-1000.0 * (kk[:, None] < kk[None, :])).astype(BF)
    wide = np.zeros((P, 896), np.float32)
    wide[kk, kk + 384] = 1.0
    wide = wide.astype(BF)
    in_maps = []
    for c in range(8):
        b, g = divmod(c, 4)
        heads = _core_heads(g)
        cols = np.concatenate([np.arange(h * D, (h + 1) * D) for h in heads])
        xt = np.ascontiguousarray(x[b].T).astype(BF)
        wq = (Wq[:, cols] * np.float32(1.0 / SQD)).astype(BF)
        wk = Wk[:, cols].astype(BF)
        wv = Wv[:, cols].astype(BF)
        wo = np.ascontiguousarray(Wo[cols, :]).astype(BF)

        # ALiBi split: key-side ramp s*(tk-center) is an exact fp32
        # per-partition exp-bias table (ktab); for the steep head positions
        # the query side -s*(tq-1024) is folded in by a rank-1 PSUM preload.
        # Row-constant rounding of qrow cancels in softmax.
        qrow = np.zeros((HPG, NQC, 512), np.float32)
        ktab = np.zeros((P, HPG, NQC, NKT), np.float32)
        p64 = np.arange(P, dtype=np.float64)
        for hi, h in enumerate(heads):
            s = SLOPES[h]
            for j in range(NQC):
                tq = 512.0 * j + np.arange(512, dtype=np.float64)
                qrow[hi, j] = (-s * (tq - 1024.0)).astype(np.float32)
                center = 1024.0 if hi <= 1 else 512.0 * j + 511.0
                for kt in range(NKT):
                    ktab[:, hi, j, kt] = (
                        s * (128.0 * kt + p64 - center)
                    ).astype(np.float32)
        qrow2 = np.zeros((33, HPG * NQC * 512), np.float32)
        qrow2[0] = qrow2[32] = qrow.reshape(-1)
        m = {
            "xt": xt, "wq": wq, "wk": wk, "wv": wv, "wo": wo,
            "qrow": qrow2.astype(BF),
            "ktab": ktab.reshape(P, HPG * NQC * NKT),
            "ones": ones, "step": step, "wide": wide,
        }
        if any(use_b):
            bqkv = np.stack([
                bq[cols] * np.float32(1.0 / SQD), bk[cols], bv[cols]
            ]).astype(BF)
            m["bqkv"] = bqkv
            m["onesrow"] = onesrow
        in_maps.append(m)
    return in_maps


def _gather(results, bo):
    out = np.zeros((B, T, C), np.float32)
    for c in range(8):
        b = c // 4
        out[b] += np.asarray(results[c]["y"], dtype=np.float32)
    out += np.asarray(bo, np.float32)[None, None, :]
    return out


def run(inputs, trace=False, tmpdir=None, trace_cores=None):
    """Full pipeline; returns (output, BassKernelResults)."""
    x = inputs["x"]
    use_b = (
        bool(np.any(inputs["bq"])),
        bool(np.any(inputs["bk"])),
        bool(np.any(inputs["bv"])),
    )
    nc = _build_program(use_b)
    in_maps = _host_inputs(
        x, inputs["Wq"], inputs["bq"], inputs["Wk"], inputs["bk"],
        inputs["Wv"], inputs["bv"], inputs["Wo"], inputs["bo"], use_b
    )
    res = run_bass_kernel_spmd(
        nc, in_maps, list(range(8)), trace=trace, tmpdir=tmpdir,
        trace_cores=trace_cores,
    )
    out = _gather(res.results, inputs["bo"])
    return out, res


def kernel(**inputs):
    out, _ = run(inputs, trace=False)
    return out


# revision 40
# speedup vs baseline: 1.0620x; 1.0054x over previous
"""Trainium2 Bass kernel for nn_CausalAttention (B=2, T=2048, C=2048, H=16, ALiBi).

Sharding: 8 cores = 2 (batch) x 4 (head groups). Core c handles batch c//4 and
heads [g, g+4, g+8, g+12] where g = c%4 (strided so the ALiBi slope mix is
balanced across cores). One SPMD program; every slope-dependent value enters
as data (exp-bias table, query-shift rows), never as a program constant.

All matmul operands are bf16 (fp32 PSUM accumulation): rel err ~4e-3 on the
final output, well inside the gate; it halves DMA/SBUF and enables the PE
fast-weight-load path. Everything is SBUF-resident - the only HBM traffic is
the inputs (x^T + weights, bf16) and the bf16 partial-output store.

Per-core device pipeline:
  A) qT/kT [d,t] and v [t,d] projections from host-pretransposed x^T by
     512-wide t-slices. DMA rings are issue-ordered: wq/xt(tn0) chunk pairs
     interleave on the HWDGE stream (tail on SWDGE), wk splits across both,
     and the four per-head accumulation chains consume kc chunks in arrival
     order, so the PE chases the transfers without idling (a short warm-up
     block on resident ones keeps the HAM clock-gate at K=8/8). Wq is
     host-prescaled by 1/sqrt(D). tn>=1 sections run [V,K,Q] so the A->B
     PSUM-bank handoff waits only on the short Q-copy tail.
  B) Per query chunk j (descending), per head: S^T[tk,tq] = kT.T @ qT in
     PSUM. ALiBi enters as (i) an exact fp32 per-partition exp-bias column
     (key-side ramp; 1024-centred for the two steep head positions,
     chunk-end-centred for the shallow two) and (ii) for the steep positions
     a query-side shift row folded in by a rank-1 matmul PSUM preload
     (softmax-invariant; range control only), packed two-at-a-time into
     distinct PE row groups via tile_position. Diagonal tiles get -1000
     accumulated on the causal triangle by a constant step x one-hot matmul,
     so ACT's exp (into SBUF bf16) yields exact zeros there - no separate
     masking pass and no cross-engine hop. PV and the denominator (all-ones
     stationary, output pre-broadcast across partitions) accumulate on the
     PE, emitted with a 3-tile software lag so the in-order PE stream never
     waits on exp. DVE only does the reciprocal + normalize per (head,
     chunk). Tiles compute only the live column range; far tiles with
     slope*(tq-tk) >= 40 everywhere are skipped (softmax weight <= e^-30 in
     the fp32 reference as well).
  C) Per chunk j, right after its 4 heads: out[t,c] partial =
     sum_h O_norm_h^T.T @ Wo_h from SBUF, stores fanned over the three DMA
     queues (HWDGE-only for the final chunk to shorten the drain tail).
Host sums the 4 head-group partials per batch and adds bo. Key bias bk
cancels in softmax; bq/bv (zero in practice) are otherwise added on-device
via K=1 outer-product matmuls.
"""

import math
import sys

sys.path.insert(0, "/opt/trn_rl_repo")

import numpy as np
import ml_dtypes

import concourse.mybir as mybir  # noqa: E402
import concourse.tile as tile  # noqa: E402
from concourse import bacc  # noqa: E402
from concourse.bass_utils import run_bass_kernel_spmd  # noqa: E402

B, T, C, H = 2, 2048, 2048, 16
D = C // H  # 128
P = 128
NKC = C // P       # 16 contraction tiles
NKT = T // P       # 16 key tiles
NQC = T // 512     # 4 query chunks of 512
HPG = 4            # heads per core
SQD = math.sqrt(D)
SKIP_CUT = 40.0  # skipped tiles have softmax weight <= e^-30: far below the gate
F32 = mybir.dt.float32
BF16 = mybir.dt.bfloat16
EXP = mybir.ActivationFunctionType.Exp
BF = ml_dtypes.bfloat16


def _slopes(n=16):
    start = 2.0 ** (-2.0 ** -(math.log2(n) - 3))
    return [start * start**i for i in range(n)]


SLOPES = _slopes(H)


def _core_heads(g):
    return [g, g + 4, g + 8, g + 12]


def _kts_for_chunk(hi, j):
    # Union over cores: the smallest slope in head-position hi is head 4*hi+3.
    s = SLOPES[4 * hi + 3]
    out = []
    for kt in range(4 * j + 4):
        mind = 512 * j - 128 * kt - 127
        if s * mind < SKIP_CUT:
            out.append(kt)
    return out


_PROG_CACHE = {}


def _build_program(use_b):
    if use_b in _PROG_CACHE:
        return _PROG_CACHE[use_b]
    use_bq, use_bk, use_bv = use_b

    nc = bacc.Bacc(None)
    xt_d = nc.declare_dram_parameter("xt", [C, T], BF16, isOutput=False)
    wq_d = nc.declare_dram_parameter("wq", [C, HPG * D], BF16, isOutput=False)
    wk_d = nc.declare_dram_parameter("wk", [C, HPG * D], BF16, isOutput=False)
    wv_d = nc.declare_dram_parameter("wv", [C, HPG * D], BF16, isOutput=False)
    wo_d = nc.declare_dram_parameter("wo", [HPG * D, C], BF16, isOutput=False)
    qrow_d = nc.declare_dram_parameter("qrow", [33, HPG * NQC * 512], BF16, isOutput=False)
    ktab_d = nc.declare_dram_parameter("ktab", [P, HPG * NQC * NKT], F32, isOutput=False)
    ones_d = nc.declare_dram_parameter("ones", [P, 512], BF16, isOutput=False)
    # causal-mask matmul constants: step[k,p] = -1000*[k<p]; wide one-hot
    # wide[k,g] = [g == k+384]. step.T @ wide[:, 384:896-off] adds -1000 on
    # the masked triangle of a diagonal S tile, so exp gives exact zeros.
    step_d = nc.declare_dram_parameter("step", [P, P], BF16, isOutput=False)
    wide_d = nc.declare_dram_parameter("wide", [P, 896], BF16, isOutput=False)
    if any(use_b):
        bqkv_d = nc.declare_dram_parameter("bqkv", [3, HPG * D], BF16, isOutput=False)
        onesrow_d = nc.declare_dram_parameter("onesrow", [1, 512], BF16, isOutput=False)
    y_d = nc.declare_dram_parameter("y", [T, C], BF16, isOutput=True)

    with tile.TileContext(nc) as tc:
        with (
            tc.tile_pool(name="perm", bufs=1) as perm,
            tc.tile_pool(name="dram", bufs=1, space="DRAM") as dpool,
        ):
            ones_sb = perm.tile([P, 512], BF16, tag="ones")
            nc.sync.dma_start(ones_sb[:], ones_d[:])
            step_sb = perm.tile([P, P], BF16, tag="step")
            nc.sync.dma_start(step_sb[:], step_d[:])
            wide_sb = perm.tile([P, 896], BF16, tag="wide")
            nc.sync.dma_start(wide_sb[:], wide_d[:])
            ktab_sb = perm.tile([P, HPG, NQC, NKT], F32, tag="ktab")
            nc.sync.dma_start(
                ktab_sb[:],
                ktab_d[:].rearrange("p (h j k) -> p h j k", h=HPG, j=NQC),
            )
            qrow_sb = perm.tile([33, HPG, NQC, 512], BF16, tag="qrow")
            nc.sync.dma_start(
                qrow_sb[:],
                qrow_d[:].rearrange("o (h j f) -> o h j f", h=HPG, j=NQC),
            )
            if any(use_b):
                bqkv_sb = perm.tile([3, HPG * D], BF16, tag="bqkv")
                onesrow_sb = perm.tile([1, 512], BF16, tag="onesrow")
                nc.sync.dma_start(bqkv_sb[:], bqkv_d[:])
                nc.sync.dma_start(onesrow_sb[:], onesrow_d[:])

            # SBUF-resident projections + attention outputs (bf16).
            qt_all = perm.tile([P, HPG, T], BF16, tag="qt")
            kt_all = perm.tile([P, HPG, T], BF16, tag="kt")
            v_all = perm.tile([P, NKT, HPG * D], BF16, tag="v")
            on_all = perm.tile([P, HPG, T], BF16, tag="on")
            wo_sb = perm.tile([P, HPG, C], BF16, tag="wo")

            # ---------------- Phase A: projections ----------------
            with (
                tc.tile_pool(name="xtp", bufs=2) as xtp,
                tc.tile_pool(name="wp", bufs=1) as wp,
                tc.tile_pool(name="psA", bufs=8, space="PSUM") as psA,
            ):
                wq_sb = wp.tile([P, NKC, HPG * D], BF16, tag="wq")
                wk_sb = wp.tile([P, NKC, HPG * D], BF16, tag="wk")
                wv_sb = wp.tile([P, NKC, HPG * D], BF16, tag="wv")
                # The scalar+sync queues share one HWDGE ring set (~190GB/s),
                # gpsimd drives SWDGE (~150GB/s); ring order is issue order.
                # Interleave wq/xt(tn0) chunk pairs so the first Q chain's
                # inputs land together, splitting the tail onto SWDGE; then
                # queue the rest on SWDGE in need-order (wk, wv, xt1-3, wo).
                xt0_sb = xtp.tile([P, NKC, 512], BF16, tag="xt")
                for kc in range(NKC):
                    eng = nc.scalar if kc < 10 else nc.gpsimd
                    eng.dma_start(wq_sb[:, kc, :], wq_d[kc * P:(kc + 1) * P, :])
                    eng.dma_start(xt0_sb[:, kc, :], xt_d[kc * P:(kc + 1) * P, 0:512])
                for kc in range(NKC):
                    # low kc on the HWDGE stream right behind the pairs; high
                    # kc leads SWDGE so the K accumulation chains never wait
                    eng = nc.scalar if kc < 8 else nc.gpsimd
                    eng.dma_start(wk_sb[:, kc, :], wk_d[kc * P:(kc + 1) * P, :])
                for kc in range(NKC):
                    nc.gpsimd.dma_start(
                        wv_sb[:, kc, :], wv_d[kc * P:(kc + 1) * P, :]
                    )

                # PE warm-up across the initial DMA window (HAM reaches
                # K=8/8 before the projection chains start), doubling as a
                # microbench: 16 N=512 then 16 N=1024 bf16 matmuls.
                wb_ps = psA.tile([P, 512], F32, tag="pp")
                for wi in range(24):
                    nc.tensor.matmul(
                        wb_ps[:], ones_sb[:, :P], ones_sb[:],
                        start=True, stop=True,
                    )
                warm_out = wp.tile([P, 512], F32, tag="wout")
                nc.vector.tensor_copy(warm_out[:], wb_ps[:])
                warm_d = dpool.tile([P, 512], F32, tag="warmd", name="warm_d")
                nc.sync.dma_start(warm_d[:], warm_out[:])

                for tn in range(NQC):
                    ts = slice(tn * 512, (tn + 1) * 512)
                    if tn == 0:
                        xt_sb = xt0_sb
                        # consume kc in DMA arrival order: SWDGE chunks land
                        # first, then the HWDGE stream, so the four
                        # interleaved chains chase the transfers without gaps
                        qorder = list(range(10, NKC)) + list(range(10))
                        korder = list(range(8, NKC)) + list(range(8))
                    else:
                        xt_sb = xtp.tile([P, NKC, 512], BF16, tag="xt")
                        nc.gpsimd.dma_start(
                            xt_sb[:], xt_d[:, ts].rearrange("(kc p) t -> p kc t", p=P)
                        )
                        qorder = korder = list(range(NKC))
                    def qk_section(w_sb, dst, ub, brow, ceng, order,
                                   split_copies=False):
                        pss = [psA.tile([P, 512], F32, tag="pp", name=f"psqk{x}")
                               for x in range(HPG)]
                        for ki, kc in enumerate(order):
                            for hi in range(HPG):
                                nc.tensor.matmul(
                                    pss[hi][:],
                                    w_sb[:, kc, hi * D:(hi + 1) * D],
                                    xt_sb[:, kc, :],
                                    start=(ki == 0),
                                    stop=(ki == NKC - 1 and not ub),
                                )
                        for hi in range(HPG):
                            if ub:
                                nc.tensor.matmul(
                                    pss[hi][:],
                                    bqkv_sb[brow:brow + 1, hi * D:(hi + 1) * D],
                                    onesrow_sb[:],
                                    start=False,
                                    stop=True,
                                )
                            if split_copies and hi % 2:
                                nc.scalar.copy(dst[:, hi, ts], pss[hi][:])
                            else:
                                ceng(dst[:, hi, ts], pss[hi][:])

                    def v_section():
                        pss = [psA.tile([P, 512], F32, tag="pp", name=f"psv{x}")
                               for x in range(4)]
                        for kc in range(NKC):
                            for tt in range(4):
                                nc.tensor.matmul(
                                    pss[tt][:],
                                    xt_sb[:, kc, tt * P:(tt + 1) * P],
                                    wv_sb[:, kc, :],
                                    start=(kc == 0),
                                    stop=(kc == NKC - 1 and not use_bv),
                                )
                        for tt in range(4):
                            gt = 4 * tn + tt
                            if use_bv:
                                nc.tensor.matmul(
                                    pss[tt][:],
                                    onesrow_sb[:, :P],
                                    bqkv_sb[2:3, :],
                                    start=False,
                                    stop=True,
                                )
                            nc.vector.tensor_copy(v_all[:, gt, :], pss[tt][:])

                    # tn0 must run [Q,K,V] (wv arrives last on SWDGE); later
                    # tns run [V,K,Q] so the A->B PSUM-bank handoff waits only
                    # on the short Q-copy tail, not the V-copy tail.
                    if tn == 0:
                        qk_section(wq_sb, qt_all, use_bq, 0,
                                   nc.vector.tensor_copy, qorder)
                        qk_section(wk_sb, kt_all, use_bk, 1,
                                   nc.scalar.copy, korder)
                        v_section()
                    else:
                        v_section()
                        qk_section(wk_sb, kt_all, use_bk, 1,
                                   nc.scalar.copy, korder)
                        qk_section(wq_sb, qt_all, use_bq, 0,
                                   nc.vector.tensor_copy, qorder,
                                   split_copies=(tn == NQC - 1))

            # wo prefetch: gpsimd queue is free from here; only needed at the
            # first phase-C block, ~10s of us away.
            for h in range(HPG):
                nc.gpsimd.dma_start(wo_sb[:, h, :], wo_d[h * P:(h + 1) * P, :])

            # ---------------- Phase B + C, fused per chunk ----------------
            with (
                tc.tile_pool(name="ep", bufs=2) as ep,
                tc.tile_pool(name="rp", bufs=2) as rp,
                tc.tile_pool(name="stC", bufs=4) as stC,
                tc.tile_pool(name="psX", bufs=4, space="PSUM") as psX,
                tc.tile_pool(name="psO", bufs=2, space="PSUM") as psO,
                tc.tile_pool(name="psD", bufs=2, space="PSUM") as psD,
            ):
                # psX serves both the S tiles (head loops) and the phase-C
                # chains (between head loops) - they never need banks at once.
                psS = psC = psX
                yqueues = [nc.sync, nc.scalar, nc.gpsimd, nc.sync]
                LAG = 3  # tiles of PV/den lag so the PE never waits on exp

                pend = []

                def emit_pending():
                    """Emit the oldest pending PV+den pair; finalize its head
                    when it is the last tile of that head's chunk."""
                    (phi, pj, pidx, pkt, pn, poff, pe_sb, po_ps, pden_ps) = pend.pop(0)
                    nc.tensor.matmul(
                        po_ps[:, poff:],
                        v_all[:, pkt, phi * D:(phi + 1) * D],
                        pe_sb[:, pidx, poff:],
                        start=(pidx == 0),
                        stop=(pidx == pn - 1),
                    )
                    nc.tensor.matmul(
                        pden_ps[:, poff:],
                        ones_sb[:, :P],
                        pe_sb[:, pidx, poff:],
                        start=(pidx == 0),
                        stop=(pidx == pn - 1),
                    )
                    if pidx == pn - 1:
                        rec = rp.tile([P, 512], F32, tag="rec", name="rec")
                        nc.vector.reciprocal_approx_fast(rec[:], pden_ps[:])
                        nc.vector.tensor_mul(
                            on_all[:, phi, pj * 512:(pj + 1) * 512],
                            po_ps[:], rec[:],
                        )

                for j in reversed(range(NQC)):
                    for hi in range(HPG):
                        kts = _kts_for_chunk(hi, j)
                        n = len(kts)
                        e_sb = ep.tile([P, NKT, 512], BF16, tag="e", name="e_sb")
                        o_ps = psO.tile([P, 512], F32, tag="op", name="o_ps")
                        den_ps = psD.tile([P, 512], F32, tag="dp", name="den_ps")
                        use_qbc = hi <= 1
                        s_ps_next = None
                        for idx, kt in enumerate(kts):
                            # live column range: diagonal tiles start later
                            off = max(0, 128 * kt - 512 * j)
                            if use_qbc:
                                # query-side shift preloaded into PSUM by a
                                # rank-1 matmul (softmax-invariant; range
                                # only); consecutive tiles share one packed
                                # PE pass via distinct row groups
                                if s_ps_next is not None:
                                    s_ps = s_ps_next
                                    s_ps_next = None
                                else:
                                    s_ps = psS.tile([P, 512], F32, tag="sp",
                                                    name="s_ps")
                                    nc.tensor.matmul(
                                        s_ps[:],
                                        ones_sb[0:1, :P],
                                        qrow_sb[0:1, hi, j, :],
                                        start=True,
                                        stop=False,
                                    )
                                    if idx + 1 < n:
                                        s_ps_next = psS.tile(
                                            [P, 512], F32, tag="sp",
                                            name="s_ps_n")
                                        nc.tensor.matmul(
                                            s_ps_next[:],
                                            ones_sb[32:33, :P],
                                            qrow_sb[32:33, hi, j, :],
                                            start=True,
                                            stop=False,
                                            tile_position=(32, 0),
                                        )
                            else:
                                s_ps = psS.tile([P, 512], F32, tag="sp",
                                                name="s_ps")
                            diag = 128 * kt > 512 * j - 128
                            nc.tensor.matmul(
                                s_ps[:, off:],
                                kt_all[:, hi, kt * P:(kt + 1) * P],
                                qt_all[:, hi, j * 512 + off:(j + 1) * 512],
                                start=not use_qbc,
                                stop=not diag,
                            )
                            if diag:
                                # accumulate -1000 on the causal triangle so
                                # exp underflows to exact zero there
                                nc.tensor.matmul(
                                    s_ps[:, off:],
                                    step_sb[:],
                                    wide_sb[:, 384:896 - off],
                                    start=False,
                                    stop=True,
                                )
                            nc.scalar.activation(
                                e_sb[:, idx, off:],
                                s_ps[:, off:],
                                EXP,
                                bias=ktab_sb[:, hi, j, kt:kt + 1],
                                scale=1.0,
                            )
                            while len(pend) > LAG:
                                emit_pending()
                            pend.append((hi, j, idx, kt, n, off, e_sb, o_ps, den_ps))
                    # drain before phase C so the PE stream stays in dep order
                    while pend:
                        emit_pending()
                    # ---- Phase C for this chunk ----
                    for tt in range(4):
                        gt = 4 * j + tt
                        tsl = slice(gt * P, (gt + 1) * P)
                        for cn in range(NQC):
                            ps = psC.tile([P, 512], F32, tag="sp")
                            for hi in range(HPG):
                                nc.tensor.matmul(
                                    ps[:],
                                    on_all[:, hi, tsl],
                                    wo_sb[:, hi, cn * 512:(cn + 1) * 512],
                                    start=(hi == 0),
                                    stop=(hi == HPG - 1),
                                )
                            st = stC.tile([P, 512], BF16, tag="st")
                            if cn % 2:
                                nc.vector.tensor_copy(st[:], ps[:])
                            else:
                                nc.scalar.copy(st[:], ps[:])
                            q = yqueues[cn] if j > 0 else (
                                nc.sync if cn % 2 else nc.scalar)
                            q.dma_start(
                                y_d[tsl, cn * 512:(cn + 1) * 512], st[:]
                            )

    nc.compile()
    _PROG_CACHE[use_b] = nc
    return nc


def _host_inputs(x, Wq, bq, Wk, bk, Wv, bv, Wo, bo, use_b):
    """Build the 8 per-core input maps."""
    x = np.asarray(x, np.float32)
    Wq = np.asarray(Wq, np.float32)
    Wk = np.asarray(Wk, np.float32)
    Wv = np.asarray(Wv, np.float32)
    Wo = np.asarray(Wo, np.float32)
    bq = np.asarray(bq, np.float32)
    bk = np.asarray(bk, np.float32)
    bv = np.asarray(bv, np.float32)

    ones = np.ones((P, 512), BF)
    onesrow = np.ones((1, 512), BF)
    kk = np.arange(P)
    step = (-1000.0 * (kk[:, None] < kk[None, :])).astype(BF)
    wide = np.zeros((P, 896), np.float32)
    wide[kk, kk + 384] = 1.0
    wide = wide.astype(BF)
    in_maps = []
    for c in range(8):
        b, g = divmod(c, 4)
        heads = _core_heads(g)
        cols = np.concatenate([np.arange(h * D, (h + 1) * D) for h in heads])
        xt = np.ascontiguousarray(x[b].T).astype(BF)
        wq = (Wq[:, cols] * np.float32(1.0 / SQD)).astype(BF)
        wk = Wk[:, cols].astype(BF)
        wv = Wv[:, cols].astype(BF)
        wo = np.ascontiguousarray(Wo[cols, :]).astype(BF)

        # ALiBi split: key-side ramp s*(tk-center) is an exact fp32
        # per-partition exp-bias table (ktab); for the steep head positions
        # the query side -s*(tq-1024) is folded in by a rank-1 PSUM preload.
        # Row-constant rounding of qrow cancels in softmax.
        qrow = np.zeros((HPG, NQC, 512), np.float32)
        ktab = np.zeros((P, HPG, NQC, NKT), np.float32)
        p64 = np.arange(P, dtype=np.float64)
        for hi, h in enumerate(heads):
            s = SLOPES[h]
            for j in range(NQC):
                tq = 512.0 * j + np.arange(512, dtype=np.float64)
                qrow[hi, j] = (-s * (tq - 1024.0)).astype(np.float32)
                center = 1024.0 if hi <= 1 else 512.0 * j + 511.0
                for kt in range(NKT):
                    ktab[:, hi, j, kt] = (
                        s * (128.0 * kt + p64 - center)
                    ).astype(np.float32)
        qrow2 = np.zeros((33, HPG * NQC * 512), np.float32)
        qrow2[0] = qrow2[32] = qrow.reshape(-1)
        m = {
            "xt": xt, "wq": wq, "wk": wk, "wv": wv, "wo": wo,
            "qrow": qrow2.astype(BF),
            "ktab": ktab.reshape(P, HPG * NQC * NKT),
            "ones": ones, "step": step, "wide": wide,
        }
        if any(use_b):
            bqkv = np.stack([
                bq[cols] * np.float32(1.0 / SQD), bk[cols], bv[cols]
            ]).astype(BF)
            m["bqkv"] = bqkv
            m["onesrow"] = onesrow
        in_maps.append(m)
    return in_maps


def _gather(results, bo):
    out = np.zeros((B, T, C), np.float32)
    for c in range(8):
        b = c // 4
        out[b] += np.asarray(results[c]["y"], dtype=np.float32)
    out += np.asarray(bo, np.float32)[None, None, :]
    return out


def run(inputs, trace=False, tmpdir=None, trace_cores=None):
    """Full pipeline; returns (output, BassKernelResults)."""
    x = inputs["x"]
    use_b = (
        bool(np.any(inputs["bq"])),
        bool(np.any(inputs["bk"])),
        bool(np.any(inputs["bv"])),
    )
    nc = _build_program(use_b)
    in_maps = _host_inputs(
        x, inputs["Wq"], inputs["bq"], inputs["Wk"], inputs["bk"],
        inputs["Wv"], inputs["bv"], inputs["Wo"], inputs["bo"], use_b
    )
    res = run_bass_kernel_spmd(
        nc, in_maps, list(range(8)), trace=trace, tmpdir=tmpdir,
        trace_cores=trace_cores,
    )
    out = _gather(res.results, inputs["bo"])
    return out, res


def kernel(**inputs):
    out, _ = run(inputs, trace=False)
    return out
